# revision 5
# baseline (speedup 1.0000x reference)
"""Trainium2 Bass kernel for LocalWindowAttention.

Computation (per batch b):
    tokens = xb[b].mean(-1)                    # [NB, C]
    Q/K/V  = tokens @ W{q,k,v} + pos           # [NB, D]
    scores = window-attn over NB (win=9, clamped) with scale 1/sqrt(D)
    ctx    = softmax(scores) @ V_window        # [NB, D]
    out    = xb[b] + (ctx @ Wo)[..., None]     # broadcast over T

Strategy: data-parallel over B across 8 NeuronCores (1 batch/core).
Per core, a single NEFF does:
  Phase A: stream xb (bf16) from HBM, T-sum on DVE -> tokens (bf16),
           PE-transpose -> tokensT.
  Proj:    Q_T/K_T (d-major) and V (n-major) via PE matmuls (bf16),
           + pos adds on DVE.
  Attn:    per 128-row block: banded scores via PE against a 136-wide
           K_T window, clamped-window softmax done exactly via an
           additive log-multiplicity mask + ACT exp (accum_out gives
           the denominator), attn transposed on PE, ctx_T and out_tok
           via PE matmuls.
  Phase B: re-stream xb (fp32), DVE broadcast-add of out_tok over T,
           store result.

The clamped gather of the reference (indices clipped at the edges,
duplicating edge rows) is reproduced exactly by adding log(multiplicity)
to the score of each unique column inside the softmax.
"""

import numpy as np
import ml_dtypes

import concourse.bass as bass
import concourse.mybir as mybir
import concourse.tile as tile
import concourse.bacc as bacc
from concourse import masks as cmasks
from concourse.bass_utils import run_bass_kernel_spmd

# Problem shapes (hardcoded per contest rules)
B, NB, C, T = 8, 1024, 1024, 32
D = 1024
WIN, HALF = 9, 4
P = 128                       # partitions
NBLK = NB // P                # 8 row blocks
CCH = C // P                  # 8 c-chunks
DCH = D // P                  # 8 d-chunks
WWIN = 192                    # window columns per block (32-aligned segs)
SCALE = 1.0 / np.sqrt(D)      # 1/32
A_CC = 128                    # phase-A c-chunk per stream tile
B_CC = 128                    # phase-B c-chunk per stream tile

F32 = mybir.dt.float32
BF16 = mybir.dt.bfloat16
F16 = mybir.dt.float16

N_CORES = 8


def _w0(i):
    """Window start for block i; chosen so all V-block segments are
    32-aligned (legal matmul tile_positions)."""
    return min(max(i * P - 32, 0), NB - WWIN)


def _build_masks():
    """Per-block additive masks [NBLK, P, WWIN] (pre-divided by SCALE):
    log(multiplicity) on in-band columns (reproduces the reference's
    clamped gather exactly), -1e30 elsewhere."""
    m = np.full((NBLK, P, WWIN), -1e30, np.float32)
    for i in range(NBLK):
        w0 = _w0(i)
        for r in range(P):
            n = i * P + r
            idx = np.clip(n - HALF + np.arange(WIN), 0, NB - 1)
            u, cnt = np.unique(idx, return_counts=True)
            m[i, r, u - w0] = np.log(cnt.astype(np.float64)) / SCALE
    return m


_MASKS = _build_masks()


def _segments(i):
    """V-block segments covering window [w0, w0+WWIN) for block i as
    (blk, p0, ln, cofs): rows [p0, p0+ln) of V block `blk` correspond to
    window columns [cofs, cofs+ln). All splits are 32-aligned so both
    the attn transposes and the banded matmuls get legal tile
    positions."""
    w0 = _w0(i)
    segs = []
    lo, hi = w0, w0 + WWIN
    for blk in range(NBLK):
        b0, b1 = blk * P, (blk + 1) * P
        s0, s1 = max(lo, b0), min(hi, b1)
        if s0 < s1:
            segs.append((blk, s0 - b0, s1 - s0, s0 - w0))
    return segs


def build_nc():
    nc = bacc.Bacc("TRN2", target_bir_lowering=False, debug=False,
                   num_devices=N_CORES)

    xb_h = nc.declare_dram_parameter("xbh", [NB, C, T], F16, isOutput=False)
    wq_d = nc.declare_dram_parameter("wq", [C, D], BF16, isOutput=False)
    wk_d = nc.declare_dram_parameter("wk", [C, D], BF16, isOutput=False)
    wv_d = nc.declare_dram_parameter("wv", [C, D], BF16, isOutput=False)
    wo_d = nc.declare_dram_parameter("wo", [D, C], BF16, isOutput=False)
    pos_d = nc.declare_dram_parameter("pos", [NB, D], BF16, isOutput=False)
    post_d = nc.declare_dram_parameter("post", [D, NB], BF16, isOutput=False)
    mask_d = nc.declare_dram_parameter("mask", [NBLK, P, WWIN], BF16,
                                       isOutput=False)
    out_d = nc.declare_dram_parameter("out", [NB, C, T], F16, isOutput=True)

    with tile.TileContext(nc) as tc:
        _emit(nc, tc, xb_h, wq_d, wk_d, wv_d, wo_d, pos_d, post_d,
              mask_d, out_d)

    nc.compile()
    return nc


def _emit(nc, tc, xb_h, wq_d, wk_d, wv_d, wo_d, pos_d, post_d,
          mask_d, out_d):
    from contextlib import ExitStack

    with ExitStack() as ctx:
        const_pool = ctx.enter_context(tc.tile_pool(name="const", bufs=1))
        ident = const_pool.tile([P, P], BF16)
        cmasks.make_identity(nc, ident[:])

        persist = ctx.enter_context(tc.tile_pool(name="persist", bufs=1))
        qT = persist.tile([P, DCH, NB], BF16)        # Q_T (d-major)
        kT = persist.tile([P, DCH, NB], BF16)        # K_T (d-major)
        vA = persist.tile([P, NBLK, D], BF16)        # V (n-major)
        mask_t = persist.tile([P, NBLK, WWIN], BF16)
        nc.sync.dma_start(mask_t[:], mask_d.rearrange("a p w -> p a w"))

        wpool = ctx.enter_context(tc.tile_pool(name="weights", bufs=1))
        wq = wpool.tile([P, CCH, D], BF16)
        wk = wpool.tile([P, CCH, D], BF16)
        wv = wpool.tile([P, CCH, D], BF16)
        wo = wpool.tile([P, DCH, C], BF16)
        nc.sync.dma_start(wq[:], wq_d.rearrange("(a p) d -> p a d", p=P))
        nc.sync.dma_start(wk[:], wk_d.rearrange("(a p) d -> p a d", p=P))
        nc.sync.dma_start(wv[:], wv_d.rearrange("(a p) d -> p a d", p=P))
        nc.sync.dma_start(wo[:], wo_d.rearrange("(a p) c -> p a c", p=P))

        stream = ctx.enter_context(tc.tile_pool(name="stream", bufs=6))
        tok_pool = ctx.enter_context(tc.tile_pool(name="tokens", bufs=1))
        tokT_pool = ctx.enter_context(tc.tile_pool(name="tokT", bufs=2))
        pos_pool = ctx.enter_context(tc.tile_pool(name="pos", bufs=1))
        otok_pool = ctx.enter_context(tc.tile_pool(name="otok", bufs=2))
        att_pool = ctx.enter_context(tc.tile_pool(name="attn", bufs=1))
        ctxT_pool = ctx.enter_context(tc.tile_pool(name="ctxT", bufs=12))

        # PSUM: 8 banks total -> 4 pools x 2 bufs, shared by size class
        psTR = ctx.enter_context(
            tc.tile_pool(name="psTR", bufs=2, space="PSUM"))  # transposes
        ps512 = ctx.enter_context(
            tc.tile_pool(name="ps512", bufs=2, space="PSUM"))  # V/out_tok
        ps128 = ctx.enter_context(
            tc.tile_pool(name="ps128", bufs=2, space="PSUM"))  # QK/ctx
        psS = ctx.enter_context(
            tc.tile_pool(name="psS", bufs=2, space="PSUM"))   # scores

        def phase_a_proj(i):
            """Stream xb block i (bf16), T-sum, transpose, project Q/K/V."""
            tok = tok_pool.tile([P, C], BF16)
            for j in range(C // A_CC):
                a = stream.tile([P, A_CC, T], F16, tag="xb")
                nc.sync.dma_start(
                    a[:], xb_h[i * P:(i + 1) * P, j * A_CC:(j + 1) * A_CC, :])
                # in-place bf16 tree-sum over T: TT adds run in DVE 2x mode
                # (vs 1x for tensor_reduce), halving the mean cost
                for h in (16, 8, 4, 2):
                    nc.vector.tensor_tensor(
                        a[:, :, 0:h], a[:, :, 0:h], a[:, :, h:2 * h],
                        op=mybir.AluOpType.add)
                with nc.allow_low_precision("tokens feed bf16 matmuls"):
                    nc.vector.tensor_reduce(
                        tok[:, j * A_CC:(j + 1) * A_CC], a[:, :, 0:2],
                        axis=mybir.AxisListType.X, op=mybir.AluOpType.add)
            tokT = tokT_pool.tile([P, CCH, P], BF16)
            for cc in range(CCH):
                pt = psTR.tile([P, P], BF16, tag="tr")
                nc.tensor.transpose(pt[:], tok[:, cc * P:(cc + 1) * P],
                                    ident[:])
                nc.scalar.copy(tokT[:, cc, :], pt[:])

            ic = slice(i * P, (i + 1) * P)
            posT_t = pos_pool.tile([P, DCH, P], BF16, tag="posT")
            nc.sync.dma_start(
                posT_t[:],
                post_d.rearrange("(a p) n -> p a n", p=P)[:, :, ic])
            for dd in range(DCH):
                for dst, w in ((qT, wq), (kT, wk)):
                    ps = ps128.tile([P, P], F32, tag="qk")
                    for cc in range(CCH):
                        nc.tensor.matmul(
                            ps[:], w[:, cc, dd * P:(dd + 1) * P],
                            tokT[:, cc, :],
                            start=(cc == 0), stop=(cc == CCH - 1))
                    nc.vector.tensor_tensor(
                        dst[:, dd, ic], ps[:], posT_t[:, dd, :],
                        op=mybir.AluOpType.add)
            posn_t = pos_pool.tile([P, D], BF16, tag="posn")
            nc.sync.dma_start(
                posn_t[:], pos_d.rearrange("(a p) d -> p a d", p=P)[:, i, :])
            for dh in range(D // 512):
                ds_ = slice(dh * 512, (dh + 1) * 512)
                ps = ps512.tile([P, 512], F32, tag="v")
                for cc in range(CCH):
                    nc.tensor.matmul(
                        ps[:], tokT[:, cc, :], wv[:, cc, ds_],
                        start=(cc == 0), stop=(cc == CCH - 1))
                nc.vector.tensor_tensor(
                    vA[:, i, ds_], ps[:], posn_t[:, ds_],
                    op=mybir.AluOpType.add)

        def attention(i):
            """Banded attention for block i -> out_tok tile, then phase B."""
            w0 = _w0(i)
            segs = _segments(i)

            sc = psS.tile([P, WWIN], F32)
            for dd in range(DCH):
                nc.tensor.matmul(sc[:], qT[:, dd, i * P:(i + 1) * P],
                                 kT[:, dd, w0:w0 + WWIN],
                                 start=(dd == 0), stop=(dd == DCH - 1))
            msk = att_pool.tile([P, WWIN], F32, tag="msk")
            nc.vector.tensor_tensor(msk[:], sc[:], mask_t[:, i, :],
                                    op=mybir.AluOpType.add)
            att = att_pool.tile([P, WWIN], F32, tag="att")
            den = att_pool.tile([P, 1], F32, tag="den")
            nc.scalar.activation(att[:], msk[:],
                                 mybir.ActivationFunctionType.Exp,
                                 scale=float(SCALE), accum_out=den[:])
            rden = att_pool.tile([P, 1], F32, tag="rden")
            nc.vector.reciprocal(rden[:], den[:])
            attb = att_pool.tile([P, WWIN], BF16, tag="attb", bufs=2)
            nc.vector.tensor_scalar_mul(attb[:], att[:], rden[:])

            # transpose attn segments (32-aligned partition placement)
            attT = []
            for (blk, p0, ln, cofs) in segs:
                pt = psTR.tile([P, P], BF16, tag="tr")
                nc.tensor.transpose(pt[p0:p0 + ln, :],
                                    attb[:, cofs:cofs + ln], ident[:],
                                    tile_position=(0, p0))
                st = att_pool.tile([P, P], BF16, tag="attT_sb", bufs=4)
                nc.scalar.copy(st[p0:p0 + ln, :], pt[p0:p0 + ln, :])
                attT.append(st)

            # ctx_T [d, n] then out_tok [n, c]
            ctxTs = []
            for dd in range(DCH):
                cps = ps128.tile([P, P], F32, tag="qk")
                for k, (blk, p0, ln, cofs) in enumerate(segs):
                    nc.tensor.matmul(
                        cps[:], vA[p0:p0 + ln, blk, dd * P:(dd + 1) * P],
                        attT[k][p0:p0 + ln, :],
                        start=(k == 0), stop=(k == len(segs) - 1),
                        tile_position=(p0, 0))
                cts = ctxT_pool.tile([P, P], BF16)
                nc.scalar.copy(cts[:], cps[:])
                ctxTs.append(cts)
            otok = otok_pool.tile([P, C], F32)
            for ch in range(C // 512):
                cs = slice(ch * 512, (ch + 1) * 512)
                ops = ps512.tile([P, 512], F32, tag="v")
                for dd in range(DCH):
                    nc.tensor.matmul(ops[:], ctxTs[dd][:], wo[:, dd, cs],
                                     start=(dd == 0), stop=(dd == DCH - 1))
                nc.scalar.copy(otok[:, cs], ops[:])
            return otok

        def phase_b(i, otok):
            """Residual broadcast-add over T for block i."""
            for j in range(C // B_CC):
                bx = stream.tile([P, B_CC, T], F16, tag="xb")
                nc.sync.dma_start(
                    bx[:], xb_h[i * P:(i + 1) * P, j * B_CC:(j + 1) * B_CC, :])
                ot = otok[:, j * B_CC:(j + 1) * B_CC]
                nc.vector.tensor_tensor(
                    bx[:], bx[:],
                    ot.unsqueeze(-1).broadcast_to((P, B_CC, T)),
                    op=mybir.AluOpType.add)
                # fp16 store (host upcasts to fp32); halves write traffic
                nc.sync.dma_start(
                    out_d[i * P:(i + 1) * P, j * B_CC:(j + 1) * B_CC, :],
                    bx[:])

        # software-pipelined emission: attention(i) needs blocks i-1..i+1;
        # keep phase-A prefetch ahead of phase-B consumption in trace order
        phase_a_proj(0)
        phase_a_proj(1)
        pend = []
        for i in range(NBLK):
            pend.append((i, attention(i)))
            if i + 2 < NBLK:
                phase_a_proj(i + 2)
            for (bi, ot) in pend:
                phase_b(bi, ot)
            pend = []


_NC = None


def _get_nc():
    global _NC
    if _NC is None:
        _NC = build_nc()
    return _NC


def _prep_in_maps(xb, Wq, Wk, Wv, Wo, pos):
    xb = np.ascontiguousarray(xb, dtype=np.float32)
    bf = ml_dtypes.bfloat16
    xb_h = xb.astype(np.float16)
    wq_h = (np.asarray(Wq, np.float32) / T).astype(bf)
    wk_h = (np.asarray(Wk, np.float32) / T).astype(bf)
    wv_h = (np.asarray(Wv, np.float32) / T).astype(bf)
    wo_h = np.asarray(Wo, np.float32).astype(bf)
    pos_h = np.asarray(pos, np.float32).astype(bf)
    post_h = np.ascontiguousarray(np.asarray(pos, np.float32).T).astype(bf)
    in_maps = []
    for b in range(B):
        in_maps.append({
            "xbh": xb_h[b],
            "wq": wq_h, "wk": wk_h, "wv": wv_h, "wo": wo_h,
            "pos": pos_h, "post": post_h, "mask": _MASKS.astype(bf),
        })
    return in_maps


def kernel(xb, Wq, Wk, Wv, Wo, pos):
    nc = _get_nc()
    in_maps = _prep_in_maps(xb, Wq, Wk, Wv, Wo, pos)
    res = run_bass_kernel_spmd(nc, in_maps, core_ids=list(range(N_CORES)))
    return np.stack([res.results[b]["out"] for b in range(B)],
                    axis=0).astype(np.float32)


def run_profiled(xb, Wq, Wk, Wv, Wo, pos, **kw):
    """Like kernel(), but NTFF-profiled; returns (out, BassKernelResults)."""
    import sys, types
    if "antenv.axon_hooks" not in sys.modules:
        try:
            from trn_agent_boot.trn_boot import _ntff_profile_via_ctypes
            hook = _ntff_profile_via_ctypes('/opt/axon/libaxon_pjrt.so')
            mod = types.ModuleType("antenv.axon_hooks")
            mod.get_axon_ntff_profile_hook = lambda: hook
            mod.set_axon_ntff_profile_hook = lambda h: None
            sys.modules["antenv.axon_hooks"] = mod
            import concourse.bass_utils as bu
            bu.upload_artifacts = lambda tmpdir: f"local:{tmpdir}"
        except Exception as e:
            print(f"profiling shim unavailable: {e}")
    nc = _get_nc()
    in_maps = _prep_in_maps(xb, Wq, Wk, Wv, Wo, pos)
    res = run_bass_kernel_spmd(nc, in_maps, core_ids=list(range(N_CORES)),
                               trace=True, **kw)
    out = np.stack([res.results[b]["out"] for b in range(B)],
                   axis=0).astype(np.float32)
    return out, res



# revision 6
# speedup vs baseline: 1.0403x; 1.0403x over previous
"""Trainium2 Bass kernel for LocalWindowAttention — v3 single-read, t-major.

Per batch b (one NeuronCore):
    tokens = xb[b].mean(-1)                    # [NB, C]
    Q/K/V  = tokens @ W{q,k,v} + pos           # [NB, D]
    scores = window-attn over NB (win=9, clamped) with scale 1/sqrt(D)
    ctx    = softmax(scores) @ V_window        # [NB, D]
    out    = xb[b] + (ctx @ Wo)[..., None]     # broadcast over T

Key ideas vs the two-pass baseline (877 us):
1. Single xb read: each 128-row block's raw chunks stay resident in
   SBUF until its out_tok is known, then the residual add happens in
   place and the chunk is stored as fp16 (host upcasts).  HBM traffic
   per core = 64 MiB in + 64 MiB out — the fp16-I/O floor.
2. T-MAJOR device layout [NB, T, C] (host pre/post-transposes): makes
   C the contiguous innermost dim of every DVE operand, so both the
   T-reduction tree and the residual broadcast-add qualify for the
   DVE 2x path (2x needs innermost stride +-1 on ALL operands; in the
   natural [NB, C, T] layout the out_tok broadcast is stride-0
   innermost and runs at 1x — measured 4.3 us/chunk vs 2.2).
   DMA stays perfect: a [P, 4, C] chunk is 8 KiB contiguous per row.
3. fp8 (e4m3) weights/Q/K/V/ctx with power-of-2 scales folded so each
   step stays single-op, and DoubleRow perf mode (256-deep
   contraction per pass) on the projection/score/out matmuls.
4. Phase B lags the stream by 2 slots so stores at slot start never
   wait on the current slot's attention chain.
Quantization: out_tok carries ~5-7% rms error but is only ~7% of
|out|; end-to-end rel err ~1.3e-3 << the 2e-2 gate.
"""

import numpy as np
import ml_dtypes

import concourse.bass as bass
import concourse.mybir as mybir
import concourse.tile as tile
import concourse.bacc as bacc
from concourse import masks as cmasks
from concourse.bass_utils import run_bass_kernel_spmd

# Problem shapes (hardcoded per contest rules)
B, NB, C, T = 8, 1024, 1024, 32
D = 1024
WIN, HALF = 9, 4
P = 128                       # partitions
NBLK = NB // P                # 8 row blocks
CCH = C // P                  # 8 c-chunks
DCH = D // P                  # 8 d-chunks
WWIN = 192                    # window columns per block (32-aligned segs)
SCALE = 1.0 / np.sqrt(D)      # 1/32
TC = 4                        # t-rows per stream chunk
NCH = T // TC                 # chunks per block (8)
NSTREAM = 17                  # stream pool buffers (2 blocks + 1 spare)

F32 = mybir.dt.float32
BF16 = mybir.dt.bfloat16
F16 = mybir.dt.float16
FP8 = mybir.dt.float8e4
NPF8 = ml_dtypes.float8_e4m3
DR = mybir.MatmulPerfMode.DoubleRow

# fp8 scale plumbing (see module docstring)
S_TOK = 1.0 / 16.0            # tokens (T-sums, std ~5.7) -> fp8
S_W = 128.0                   # projection weights W/T -> fp8
S_QK = S_TOK * S_W            # = 8: Q/K/V psum pre-scale
S_O = 16.0                    # Wo -> fp8

N_CORES = 8


def _w0(i):
    """Window start for block i; all V-block segments 32-aligned."""
    return min(max(i * P - 32, 0), NB - WWIN)


def _build_masks():
    """Per-block additive masks [NBLK, P, WWIN], pre-scaled for the
    exp (which uses scale=SCALE/S_QK^2): log(multiplicity) *
    S_QK^2/SCALE on in-band columns, -1e30 elsewhere."""
    m = np.full((NBLK, P, WWIN), -1e30, np.float32)
    for i in range(NBLK):
        w0 = _w0(i)
        for r in range(P):
            n = i * P + r
            idx = np.clip(n - HALF + np.arange(WIN), 0, NB - 1)
            u, cnt = np.unique(idx, return_counts=True)
            m[i, r, u - w0] = np.log(cnt.astype(np.float64)) * (
                S_QK * S_QK / SCALE)
    return m


_MASKS64 = _build_masks()


def _segments(i):
    """V-block segments covering window [w0, w0+WWIN) for block i as
    (blk, p0, ln, cofs): rows [p0, p0+ln) of V block `blk` correspond
    to window columns [cofs, cofs+ln).  All splits 32-aligned."""
    w0 = _w0(i)
    segs = []
    lo, hi = w0, w0 + WWIN
    for blk in range(NBLK):
        b0, b1 = blk * P, (blk + 1) * P
        s0, s1 = max(lo, b0), min(hi, b1)
        if s0 < s1:
            segs.append((blk, s0 - b0, s1 - s0, s0 - w0))
    return segs


def build_nc():
    nc = bacc.Bacc("TRN2", target_bir_lowering=False, debug=False,
                   num_devices=N_CORES)

    xb_h = nc.declare_dram_parameter("xbh", [NB, T, C], F16, isOutput=False)
    wq_d = nc.declare_dram_parameter("wq", [C, D], FP8, isOutput=False)
    wk_d = nc.declare_dram_parameter("wk", [C, D], FP8, isOutput=False)
    wv_d = nc.declare_dram_parameter("wv", [C, D], FP8, isOutput=False)
    wo_d = nc.declare_dram_parameter("wo", [D, C], FP8, isOutput=False)
    posn_d = nc.declare_dram_parameter("posn", [NB, D], FP8, isOutput=False)
    post_d = nc.declare_dram_parameter("post", [D, NB], FP8, isOutput=False)
    mask_d = nc.declare_dram_parameter("mask", [NBLK, P, WWIN], BF16,
                                       isOutput=False)
    out_d = nc.declare_dram_parameter("out", [NB, T, C], F16, isOutput=True)

    with tile.TileContext(nc) as tc:
        _emit(nc, tc, xb_h, wq_d, wk_d, wv_d, wo_d, posn_d, post_d,
              mask_d, out_d)

    nc.compile()
    return nc


def _emit(nc, tc, xb_h, wq_d, wk_d, wv_d, wo_d, posn_d, post_d,
          mask_d, out_d):
    from contextlib import ExitStack

    with ExitStack() as ctx:
        const_pool = ctx.enter_context(tc.tile_pool(name="const", bufs=1))
        ident = const_pool.tile([P, P], BF16)
        cmasks.make_identity(nc, ident[:])

        persist = ctx.enter_context(tc.tile_pool(name="persist", bufs=1))
        kT = persist.tile([P, DCH, NB], FP8)         # K^T (d-major), *8
        # U = (V@Wo) (n-major, *8), rolling window of 4 blocks: lets
        # attention finish with a single attn^T @ U matmul instead of
        # the two-stage (attn@V)@Wo, shortening the per-slot critical
        # chain by ~6us
        uA = persist.tile([P, 4, C], FP8)
        mask_t = persist.tile([P, NBLK, WWIN], BF16)
        nc.sync.dma_start(mask_t[:], mask_d.rearrange("a p w -> p a w"))

        wpool = ctx.enter_context(tc.tile_pool(name="weights", bufs=1))
        wq = wpool.tile([P, CCH, D], FP8)
        wk = wpool.tile([P, CCH, D], FP8)
        wv = wpool.tile([P, CCH, D], FP8)
        wo = wpool.tile([P, DCH, C], FP8)
        nc.sync.dma_start(wq[:], wq_d.rearrange("(a p) d -> p a d", p=P))
        nc.sync.dma_start(wk[:], wk_d.rearrange("(a p) d -> p a d", p=P))
        nc.sync.dma_start(wv[:], wv_d.rearrange("(a p) d -> p a d", p=P))
        nc.sync.dma_start(wo[:], wo_d.rearrange("(a p) c -> p a c", p=P))

        stream = ctx.enter_context(tc.tile_pool(name="stream", bufs=NSTREAM))
        scr_pool = ctx.enter_context(tc.tile_pool(name="scratch", bufs=1))
        tok_pool = ctx.enter_context(tc.tile_pool(name="tokens", bufs=1))
        tokT_pool = ctx.enter_context(tc.tile_pool(name="tokT", bufs=2))
        qT_pool = ctx.enter_context(tc.tile_pool(name="qT", bufs=2))
        pos_pool = ctx.enter_context(tc.tile_pool(name="pos", bufs=1))
        otok_pool = ctx.enter_context(tc.tile_pool(name="otok", bufs=2))
        att_pool = ctx.enter_context(tc.tile_pool(name="attn", bufs=1))
        vblk_pool = ctx.enter_context(tc.tile_pool(name="vblk", bufs=1))
        vT_pool = ctx.enter_context(tc.tile_pool(name="vT", bufs=1))

        # PSUM: 8 banks -> 4 pools x 2 bufs
        psTR = ctx.enter_context(
            tc.tile_pool(name="psTR", bufs=2, space="PSUM"))  # transposes
        ps512 = ctx.enter_context(
            tc.tile_pool(name="ps512", bufs=2, space="PSUM"))  # V/out_tok
        ps128 = ctx.enter_context(
            tc.tile_pool(name="ps128", bufs=2, space="PSUM"))  # QK/ctx
        psS = ctx.enter_context(
            tc.tile_pool(name="psS", bufs=2, space="PSUM"))   # scores

        def load_sum_kv(i):
            """Stream block i (kept resident), T-sum, transpose,
            project K and V (which gate attention(i-1))."""
            tiles = []
            tok = tok_pool.tile([P, C], BF16)
            for j in range(NCH):
                t = stream.tile([P, TC, C], F16, tag="xb")
                nc.sync.dma_start(
                    t[:], xb_h[i * P:(i + 1) * P, j * TC:(j + 1) * TC, :])
                tiles.append(t)
                # all-2x reduction tree: out-of-place first level keeps
                # the raw xb intact for the phase-B residual
                s = scr_pool.tile([P, 2, C], F16, tag="s")
                nc.vector.tensor_tensor(
                    s[:], t[:, 0:2, :], t[:, 2:4, :],
                    op=mybir.AluOpType.add)
                with nc.allow_low_precision("tokens feed fp8 matmuls"):
                    if j == 0:
                        nc.vector.tensor_tensor(
                            tok[:], s[:, 0, :], s[:, 1, :],
                            op=mybir.AluOpType.add)
                    else:
                        nc.vector.tensor_tensor(
                            s[:, 0, :], s[:, 0, :], s[:, 1, :],
                            op=mybir.AluOpType.add)
                        nc.vector.tensor_tensor(
                            tok[:], tok[:], s[:, 0, :],
                            op=mybir.AluOpType.add)
            tokT = tokT_pool.tile([P, CCH, P], FP8)
            for cc in range(CCH):
                pt = psTR.tile([P, P], BF16, tag="tr")
                nc.tensor.transpose(pt[:], tok[:, cc * P:(cc + 1) * P],
                                    ident[:])
                nc.scalar.activation(tokT[:, cc, :], pt[:],
                                     mybir.ActivationFunctionType.Copy,
                                     scale=float(S_TOK))

            ic = slice(i * P, (i + 1) * P)
            posT8 = pos_pool.tile([P, DCH, P], FP8, tag="posT")
            nc.sync.dma_start(
                posT8[:],
                post_d.rearrange("(a p) n -> p a n", p=P)[:, :, ic])
            posn8 = pos_pool.tile([P, D], FP8, tag="posn")
            nc.sync.dma_start(
                posn8[:], posn_d.rearrange("(a p) d -> p a d", p=P)[:, i, :])

            # K first: attention(i-1) waits on it. fp8 DoubleRow: 256
            # contraction rows per pass.
            for dd in range(DCH):
                ps = ps128.tile([P, P], F32, tag="qk")
                for cc in range(0, CCH, 2):
                    nc.tensor.matmul(
                        ps[:], wk[:, cc:cc + 2, dd * P:(dd + 1) * P],
                        tokT[:, cc:cc + 2, :],
                        start=(cc == 0), stop=(cc == CCH - 2),
                        perf_mode=DR)
                nc.vector.tensor_tensor(kT[:, dd, ic], ps[:],
                                        posT8[:, dd, :],
                                        op=mybir.AluOpType.add)
            vblk = vblk_pool.tile([P, D], BF16)      # 8*(V+pos)
            for dh in range(D // 512):
                ds_ = slice(dh * 512, (dh + 1) * 512)
                ps = ps512.tile([P, 512], F32, tag="v")
                for cc in range(0, CCH, 2):
                    nc.tensor.matmul(
                        ps[:], tokT[:, cc:cc + 2, :], wv[:, cc:cc + 2, ds_],
                        start=(cc == 0), stop=(cc == CCH - 2),
                        perf_mode=DR)
                nc.vector.tensor_tensor(vblk[:, ds_], ps[:],
                                        posn8[:, ds_],
                                        op=mybir.AluOpType.add)
            # U(i) = V@Wo via d-major transpose of V; psum = 128*V@Wo
            # -> evict *1/16 keeps U at *8 in fp8
            vT = vT_pool.tile([P, DCH, P], FP8)
            for dd in range(DCH):
                pt = psTR.tile([P, P], BF16, tag="tr")
                nc.tensor.transpose(pt[:], vblk[:, dd * P:(dd + 1) * P],
                                    ident[:])
                nc.scalar.copy(vT[:, dd, :], pt[:])
            for ch in range(C // 512):
                cs = slice(ch * 512, (ch + 1) * 512)
                ups = ps512.tile([P, 512], F32, tag="v")
                for dd in range(0, DCH, 2):
                    nc.tensor.matmul(ups[:], vT[:, dd:dd + 2, :],
                                     wo[:, dd:dd + 2, cs],
                                     start=(dd == 0), stop=(dd == DCH - 2),
                                     perf_mode=DR)
                nc.scalar.activation(uA[:, i % 4, cs], ups[:],
                                     mybir.ActivationFunctionType.Copy,
                                     scale=float(1.0 / S_O))
            return tiles, tokT, posT8

        def proj_q(i, tokT, posT8):
            """Q projection for block i (needed only by attention(i),
            so emitted after attention(i-1) to keep PE unblocked)."""
            qTb = qT_pool.tile([P, DCH, P], FP8)
            for dd in range(DCH):
                ps = ps128.tile([P, P], F32, tag="qk")
                for cc in range(0, CCH, 2):
                    nc.tensor.matmul(
                        ps[:], wq[:, cc:cc + 2, dd * P:(dd + 1) * P],
                        tokT[:, cc:cc + 2, :],
                        start=(cc == 0), stop=(cc == CCH - 2),
                        perf_mode=DR)
                nc.vector.tensor_tensor(qTb[:, dd, :], ps[:],
                                        posT8[:, dd, :],
                                        op=mybir.AluOpType.add)
            return qTb

        def attention(x, qTb):
            """Banded attention for block x -> out_tok tile [P, C]."""
            w0 = _w0(x)
            segs = _segments(x)

            sc = psS.tile([P, WWIN], F32)
            for dd in range(0, DCH, 2):
                nc.tensor.matmul(sc[:], qTb[:, dd:dd + 2, :],
                                 kT[:, dd:dd + 2, w0:w0 + WWIN],
                                 start=(dd == 0), stop=(dd == DCH - 2),
                                 perf_mode=DR)
            msk = att_pool.tile([P, WWIN], F32, tag="msk")
            nc.vector.tensor_tensor(msk[:], sc[:], mask_t[:, x, :],
                                    op=mybir.AluOpType.add)
            att = att_pool.tile([P, WWIN], F32, tag="att")
            den = att_pool.tile([P, 1], F32, tag="den")
            nc.scalar.activation(att[:], msk[:],
                                 mybir.ActivationFunctionType.Exp,
                                 scale=float(SCALE / (S_QK * S_QK)),
                                 accum_out=den[:])
            rden = att_pool.tile([P, 1], F32, tag="rden")
            nc.vector.reciprocal(rden[:], den[:])
            attb = att_pool.tile([P, WWIN], BF16, tag="attb")
            nc.vector.tensor_scalar_mul(attb[:], att[:], rden[:])

            # transpose attn segments (32-aligned partition placement)
            attT = []
            for (blk, p0, ln, cofs) in segs:
                pt = psTR.tile([P, P], BF16, tag="tr")
                nc.tensor.transpose(pt[p0:p0 + ln, :],
                                    attb[:, cofs:cofs + ln], ident[:],
                                    tile_position=(0, p0))
                st = att_pool.tile([P, P], FP8, tag="attT_sb", bufs=4)
                nc.scalar.copy(st[p0:p0 + ln, :], pt[p0:p0 + ln, :])
                attT.append(st)

            # out_tok = attn^T-weighted sum of U rows; psum = 8*out_tok
            otok = otok_pool.tile([P, C], BF16)
            for ch in range(C // 512):
                cs = slice(ch * 512, (ch + 1) * 512)
                ops = ps512.tile([P, 512], F32, tag="v")
                for k, (blk, p0, ln, cofs) in enumerate(segs):
                    nc.tensor.matmul(
                        ops[:], attT[k][p0:p0 + ln, :],
                        uA[p0:p0 + ln, blk % 4, cs],
                        start=(k == 0), stop=(k == len(segs) - 1),
                        tile_position=(p0, 0))
                nc.scalar.activation(otok[:, cs], ops[:],
                                     mybir.ActivationFunctionType.Copy,
                                     scale=float(1.0 / S_QK))
            return otok

        def phase_b(x, tiles, otok):
            """In-place residual broadcast-add on the resident xb
            chunks of block x, then fp16 store (ACT HWDGE ring so
            stores never head-of-line-block the SP-ring loads).
            t-major layout puts the broadcast on an outer dim, so the
            adds run on the DVE 2x path (~2.2us/chunk)."""
            for j in range(NCH):
                t = tiles[j]
                nc.vector.tensor_tensor(
                    t[:], t[:],
                    otok.unsqueeze(1).broadcast_to((P, TC, C)),
                    op=mybir.AluOpType.add)
                nc.scalar.dma_start(
                    out_d[x * P:(x + 1) * P, j * TC:(j + 1) * TC, :], t[:])

        # Software pipeline, phase B lagged TWO slots behind the loads:
        # at slot i the stores of block i-2 (whose out_tok was computed
        # during slot i-1) begin immediately, so the DMA engines never
        # wait on the current slot's attention chain.
        prev = None        # (x, tiles, qTb): awaiting attention
        pend = None        # (x, tiles, otok): awaiting phase B
        for i in range(NBLK):
            if pend is not None:
                phase_b(*pend)
                pend = None
            tiles_i, tokT_i, posT8_i = load_sum_kv(i)
            if prev is not None:
                x, tiles_x, qTb_x = prev
                otok_x = attention(x, qTb_x)
                pend = (x, tiles_x, otok_x)
            qTb_i = proj_q(i, tokT_i, posT8_i)
            prev = (i, tiles_i, qTb_i)
        if pend is not None:
            phase_b(*pend)
        x, tiles_x, qTb_x = prev
        otok_x = attention(x, qTb_x)
        phase_b(x, tiles_x, otok_x)


_NC = None


def _get_nc():
    global _NC
    if _NC is None:
        _NC = build_nc()
    return _NC


def _prep_in_maps(xb, Wq, Wk, Wv, Wo, pos):
    # device works t-major: [NB, T, C]
    xb_h = np.asarray(xb, np.float32).transpose(0, 1, 3, 2).astype(
        np.float16)
    wq8 = (np.asarray(Wq, np.float32) * (S_W / T)).astype(NPF8)
    wk8 = (np.asarray(Wk, np.float32) * (S_W / T)).astype(NPF8)
    wv8 = (np.asarray(Wv, np.float32) * (S_W / T)).astype(NPF8)
    wo8 = (np.asarray(Wo, np.float32) * S_O).astype(NPF8)
    posn8 = (np.asarray(pos, np.float32) * S_QK).astype(NPF8)
    post8 = np.ascontiguousarray(posn8.T)
    mask_h = _MASKS64.astype(ml_dtypes.bfloat16)
    in_maps = []
    for b in range(B):
        in_maps.append({
            "xbh": np.ascontiguousarray(xb_h[b]),
            "wq": wq8, "wk": wk8, "wv": wv8, "wo": wo8,
            "posn": posn8, "post": post8, "mask": mask_h,
        })
    return in_maps


def _post(res):
    out = np.stack([res.results[b]["out"] for b in range(B)], axis=0)
    # [B, NB, T, C] fp16 -> [B, NB, C, T] fp32
    return np.ascontiguousarray(out.transpose(0, 1, 3, 2)).astype(
        np.float32)


def kernel(xb, Wq, Wk, Wv, Wo, pos):
    nc = _get_nc()
    in_maps = _prep_in_maps(xb, Wq, Wk, Wv, Wo, pos)
    res = run_bass_kernel_spmd(nc, in_maps, core_ids=list(range(N_CORES)))
    return _post(res)


def run_profiled(xb, Wq, Wk, Wv, Wo, pos, **kw):
    """Like kernel(), but NTFF-profiled; returns (out, BassKernelResults)."""
    import sys, types
    if "antenv.axon_hooks" not in sys.modules:
        try:
            from trn_agent_boot.trn_boot import _ntff_profile_via_ctypes
            hook = _ntff_profile_via_ctypes('/opt/axon/libaxon_pjrt.so')
            mod = types.ModuleType("antenv.axon_hooks")
            mod.get_axon_ntff_profile_hook = lambda: hook
            mod.set_axon_ntff_profile_hook = lambda h: None
            sys.modules["antenv.axon_hooks"] = mod
            import concourse.bass_utils as bu
            bu.upload_artifacts = lambda tmpdir: f"local:{tmpdir}"
        except Exception as e:
            print(f"profiling shim unavailable: {e}")
    nc = _get_nc()
    in_maps = _prep_in_maps(xb, Wq, Wk, Wv, Wo, pos)
    res = run_bass_kernel_spmd(nc, in_maps, core_ids=list(range(N_CORES)),
                               trace=True, **kw)
    return _post(res), res


# revision 7
# speedup vs baseline: 1.1699x; 1.1246x over previous
"""Trainium2 Bass kernel for LocalWindowAttention — v3 single-read, t-major.

Per batch b (one NeuronCore):
    tokens = xb[b].mean(-1)                    # [NB, C]
    Q/K/V  = tokens @ W{q,k,v} + pos           # [NB, D]
    scores = window-attn over NB (win=9, clamped) with scale 1/sqrt(D)
    ctx    = softmax(scores) @ V_window        # [NB, D]
    out    = xb[b] + (ctx @ Wo)[..., None]     # broadcast over T

Key ideas vs the two-pass baseline (877 us):
1. Single xb read: each 128-row block's raw chunks stay resident in
   SBUF until its out_tok is known, then the residual add happens in
   place and the chunk is stored as fp16 (host upcasts).  HBM traffic
   per core = 64 MiB in + 64 MiB out — the fp16-I/O floor.
2. T-MAJOR device layout [NB, T, C] (host pre/post-transposes): makes
   C the contiguous innermost dim of every DVE operand, so both the
   T-reduction tree and the residual broadcast-add qualify for the
   DVE 2x path (2x needs innermost stride +-1 on ALL operands; in the
   natural [NB, C, T] layout the out_tok broadcast is stride-0
   innermost and runs at 1x — measured 4.3 us/chunk vs 2.2).
   DMA stays perfect: a [P, 4, C] chunk is 8 KiB contiguous per row.
3. fp8 (e4m3) weights/Q/K/V/ctx with power-of-2 scales folded so each
   step stays single-op, and DoubleRow perf mode (256-deep
   contraction per pass) on the projection/score/out matmuls.
4. Phase B lags the stream by 2 slots so stores at slot start never
   wait on the current slot's attention chain.
Quantization: out_tok carries ~5-7% rms error but is only ~7% of
|out|; end-to-end rel err ~1.3e-3 << the 2e-2 gate.
"""

import numpy as np
import ml_dtypes

import concourse.bass as bass
import concourse.mybir as mybir
import concourse.tile as tile
import concourse.bacc as bacc
from concourse import masks as cmasks
from concourse.bass_utils import run_bass_kernel_spmd

# Problem shapes (hardcoded per contest rules)
B, NB, C, T = 8, 1024, 1024, 32
D = 1024
WIN, HALF = 9, 4
P = 128                       # partitions
NBLK = NB // P                # 8 row blocks
CCH = C // P                  # 8 c-chunks
DCH = D // P                  # 8 d-chunks
WWIN = 192                    # window columns per block (32-aligned segs)
SCALE = 1.0 / np.sqrt(D)      # 1/32
TC = 4                        # t-rows per stream chunk
NCH = T // TC                 # chunks per block (8)
NSTREAM = 18                  # stream pool buffers (2 blocks + 2 spares)

F32 = mybir.dt.float32
BF16 = mybir.dt.bfloat16
F16 = mybir.dt.float16
FP8 = mybir.dt.float8e4
NPF8 = ml_dtypes.float8_e4m3
DR = mybir.MatmulPerfMode.DoubleRow

# fp8 scale plumbing (see module docstring)
S_TOK = 1.0 / 16.0            # tokens (T-sums, std ~5.7) -> fp8
S_W = 128.0                   # projection weights W/T -> fp8
S_QK = S_TOK * S_W            # = 8: Q/K/V psum pre-scale
S_WVO = 128.0                 # fused (Wv/T)@Wo -> fp8

N_CORES = 8


def _w0(i):
    """Window start for block i; all V-block segments 32-aligned."""
    return min(max(i * P - 32, 0), NB - WWIN)


def _build_masks():
    """Per-block additive masks [NBLK, P, WWIN], pre-scaled for the
    exp (which uses scale=SCALE/S_QK^2): log(multiplicity) *
    S_QK^2/SCALE on in-band columns, -1e30 elsewhere."""
    m = np.full((NBLK, P, WWIN), -1e30, np.float32)
    for i in range(NBLK):
        w0 = _w0(i)
        for r in range(P):
            n = i * P + r
            idx = np.clip(n - HALF + np.arange(WIN), 0, NB - 1)
            u, cnt = np.unique(idx, return_counts=True)
            m[i, r, u - w0] = np.log(cnt.astype(np.float64)) * (
                S_QK * S_QK / SCALE)
    return m


_MASKS64 = _build_masks()


def _segments(i):
    """V-block segments covering window [w0, w0+WWIN) for block i as
    (blk, p0, ln, cofs): rows [p0, p0+ln) of V block `blk` correspond
    to window columns [cofs, cofs+ln).  All splits 32-aligned."""
    w0 = _w0(i)
    segs = []
    lo, hi = w0, w0 + WWIN
    for blk in range(NBLK):
        b0, b1 = blk * P, (blk + 1) * P
        s0, s1 = max(lo, b0), min(hi, b1)
        if s0 < s1:
            segs.append((blk, s0 - b0, s1 - s0, s0 - w0))
    return segs


def build_nc():
    nc = bacc.Bacc("TRN2", target_bir_lowering=False, debug=False,
                   num_devices=N_CORES)

    xb_h = nc.declare_dram_parameter("xbh", [NB, T, C], F16, isOutput=False)
    wq_d = nc.declare_dram_parameter("wq", [C, D], FP8, isOutput=False)
    wk_d = nc.declare_dram_parameter("wk", [C, D], FP8, isOutput=False)
    wvo_d = nc.declare_dram_parameter("wvo", [C, C], FP8, isOutput=False)
    posu_d = nc.declare_dram_parameter("posu", [NB, C], FP8, isOutput=False)
    post_d = nc.declare_dram_parameter("post", [D, NB], FP8, isOutput=False)
    mask_d = nc.declare_dram_parameter("mask", [NBLK, P, WWIN], BF16,
                                       isOutput=False)
    out_d = nc.declare_dram_parameter("out", [NB, T, C], F16, isOutput=True)

    with tile.TileContext(nc) as tc:
        _emit(nc, tc, xb_h, wq_d, wk_d, wvo_d, posu_d, post_d,
              mask_d, out_d)

    nc.compile()
    return nc


def _emit(nc, tc, xb_h, wq_d, wk_d, wvo_d, posu_d, post_d,
          mask_d, out_d):
    from contextlib import ExitStack

    with ExitStack() as ctx:
        const_pool = ctx.enter_context(tc.tile_pool(name="const", bufs=1))
        ident = const_pool.tile([P, P], BF16)
        cmasks.make_identity(nc, ident[:])

        persist = ctx.enter_context(tc.tile_pool(name="persist", bufs=1))
        kT = persist.tile([P, DCH, NB], FP8)         # K^T (d-major), *8
        # U = (V@Wo) (n-major, *8), rolling window of 4 blocks: lets
        # attention finish with a single attn^T @ U matmul instead of
        # the two-stage (attn@V)@Wo, shortening the per-slot critical
        # chain by ~6us
        uA = persist.tile([P, 4, C], FP8)
        mask_t = persist.tile([P, NBLK, WWIN], BF16)
        nc.sync.dma_start(mask_t[:], mask_d.rearrange("a p w -> p a w"))

        wpool = ctx.enter_context(tc.tile_pool(name="weights", bufs=1))
        wq = wpool.tile([P, CCH, D], FP8)
        wk = wpool.tile([P, CCH, D], FP8)
        wvo = wpool.tile([P, CCH, C], FP8)
        nc.sync.dma_start(wq[:], wq_d.rearrange("(a p) d -> p a d", p=P))
        nc.sync.dma_start(wk[:], wk_d.rearrange("(a p) d -> p a d", p=P))
        nc.sync.dma_start(wvo[:], wvo_d.rearrange("(a p) c -> p a c", p=P))

        stream = ctx.enter_context(tc.tile_pool(name="stream", bufs=NSTREAM))
        scr_pool = ctx.enter_context(tc.tile_pool(name="scratch", bufs=1))
        tok_pool = ctx.enter_context(tc.tile_pool(name="tokens", bufs=1))
        tokT_pool = ctx.enter_context(tc.tile_pool(name="tokT", bufs=2))
        qT_pool = ctx.enter_context(tc.tile_pool(name="qT", bufs=2))
        pos_pool = ctx.enter_context(tc.tile_pool(name="pos", bufs=1))
        otok_pool = ctx.enter_context(tc.tile_pool(name="otok", bufs=2))
        att_pool = ctx.enter_context(tc.tile_pool(name="attn", bufs=1))

        # PSUM: 8 banks -> 4 pools x 2 bufs
        psTR = ctx.enter_context(
            tc.tile_pool(name="psTR", bufs=2, space="PSUM"))  # transposes
        ps512 = ctx.enter_context(
            tc.tile_pool(name="ps512", bufs=2, space="PSUM"))  # V/out_tok
        ps128 = ctx.enter_context(
            tc.tile_pool(name="ps128", bufs=2, space="PSUM"))  # QK/ctx
        psS = ctx.enter_context(
            tc.tile_pool(name="psS", bufs=2, space="PSUM"))   # scores

        def load_sum_kv(i):
            """Stream block i (kept resident), T-sum, transpose,
            project K and V (which gate attention(i-1))."""
            tiles = []
            tok = tok_pool.tile([P, C], BF16)
            for j in range(NCH):
                t = stream.tile([P, TC, C], F16, tag="xb")
                nc.sync.dma_start(
                    t[:], xb_h[i * P:(i + 1) * P, j * TC:(j + 1) * TC, :])
                tiles.append(t)
                # all-2x reduction tree: out-of-place first level keeps
                # the raw xb intact for the phase-B residual
                s = scr_pool.tile([P, 2, C], F16, tag="s")
                nc.vector.tensor_tensor(
                    s[:], t[:, 0:2, :], t[:, 2:4, :],
                    op=mybir.AluOpType.add)
                with nc.allow_low_precision("tokens feed fp8 matmuls"):
                    if j == 0:
                        nc.vector.tensor_tensor(
                            tok[:], s[:, 0, :], s[:, 1, :],
                            op=mybir.AluOpType.add)
                    else:
                        nc.vector.tensor_tensor(
                            s[:, 0, :], s[:, 0, :], s[:, 1, :],
                            op=mybir.AluOpType.add)
                        nc.vector.tensor_tensor(
                            tok[:], tok[:], s[:, 0, :],
                            op=mybir.AluOpType.add)
            tokT = tokT_pool.tile([P, CCH, P], FP8)
            for cc in range(CCH):
                pt = psTR.tile([P, P], BF16, tag="tr")
                nc.tensor.transpose(pt[:], tok[:, cc * P:(cc + 1) * P],
                                    ident[:])
                nc.scalar.activation(tokT[:, cc, :], pt[:],
                                     mybir.ActivationFunctionType.Copy,
                                     scale=float(S_TOK))

            ic = slice(i * P, (i + 1) * P)
            posT8 = pos_pool.tile([P, DCH, P], FP8, tag="posT")
            nc.sync.dma_start(
                posT8[:],
                post_d.rearrange("(a p) n -> p a n", p=P)[:, :, ic])
            posu8 = pos_pool.tile([P, C], FP8, tag="posu")
            nc.sync.dma_start(
                posu8[:], posu_d.rearrange("(a p) c -> p a c", p=P)[:, i, :])

            # K first: attention(i-1) waits on it. fp8 DoubleRow: 256
            # contraction rows per pass.
            for dd in range(DCH):
                ps = ps128.tile([P, P], F32, tag="qk")
                for cc in range(0, CCH, 2):
                    nc.tensor.matmul(
                        ps[:], wk[:, cc:cc + 2, dd * P:(dd + 1) * P],
                        tokT[:, cc:cc + 2, :],
                        start=(cc == 0), stop=(cc == CCH - 2),
                        perf_mode=DR)
                nc.vector.tensor_tensor(kT[:, dd, ic], ps[:],
                                        posT8[:, dd, :],
                                        op=mybir.AluOpType.add)
            return tiles, tokT, posT8, posu8

        def proj_u(i, tokT, posu8):
            """U(i) = tokens@((Wv/T)@Wo) + pos@Wo, fused on the host
            into one [C, C] weight — psum arrives at *8 like Q/K, one
            DVE add of posu finishes it.  Emitted between attention's
            softmax and its out_tok matmul so the PE fills the softmax
            latency."""
            for ch in range(C // 512):
                cs = slice(ch * 512, (ch + 1) * 512)
                ups = ps512.tile([P, 512], F32, tag="v")
                for cc in range(0, CCH, 2):
                    nc.tensor.matmul(ups[:], tokT[:, cc:cc + 2, :],
                                     wvo[:, cc:cc + 2, cs],
                                     start=(cc == 0), stop=(cc == CCH - 2),
                                     perf_mode=DR)
                nc.vector.tensor_tensor(uA[:, i % 4, cs], ups[:],
                                        posu8[:, cs],
                                        op=mybir.AluOpType.add)

        def proj_q(i, tokT, posT8):
            """Q projection for block i (needed only by attention(i),
            so emitted after attention(i-1) to keep PE unblocked)."""
            qTb = qT_pool.tile([P, DCH, P], FP8)
            for dd in range(DCH):
                ps = ps128.tile([P, P], F32, tag="qk")
                for cc in range(0, CCH, 2):
                    nc.tensor.matmul(
                        ps[:], wq[:, cc:cc + 2, dd * P:(dd + 1) * P],
                        tokT[:, cc:cc + 2, :],
                        start=(cc == 0), stop=(cc == CCH - 2),
                        perf_mode=DR)
                nc.vector.tensor_tensor(qTb[:, dd, :], ps[:],
                                        posT8[:, dd, :],
                                        op=mybir.AluOpType.add)
            return qTb

        def attn_scores(x, qTb):
            """Banded scores + softmax + transposed attn for block x."""
            w0 = _w0(x)
            segs = _segments(x)

            sc = psS.tile([P, WWIN], F32)
            for dd in range(0, DCH, 2):
                nc.tensor.matmul(sc[:], qTb[:, dd:dd + 2, :],
                                 kT[:, dd:dd + 2, w0:w0 + WWIN],
                                 start=(dd == 0), stop=(dd == DCH - 2),
                                 perf_mode=DR)
            msk = att_pool.tile([P, WWIN], F32, tag="msk")
            nc.vector.tensor_tensor(msk[:], sc[:], mask_t[:, x, :],
                                    op=mybir.AluOpType.add)
            att = att_pool.tile([P, WWIN], F32, tag="att")
            den = att_pool.tile([P, 1], F32, tag="den")
            nc.scalar.activation(att[:], msk[:],
                                 mybir.ActivationFunctionType.Exp,
                                 scale=float(SCALE / (S_QK * S_QK)),
                                 accum_out=den[:])
            rden = att_pool.tile([P, 1], F32, tag="rden")
            nc.vector.reciprocal(rden[:], den[:])
            attb = att_pool.tile([P, WWIN], BF16, tag="attb")
            nc.vector.tensor_scalar_mul(attb[:], att[:], rden[:])

            # transpose attn segments (32-aligned partition placement)
            attT = []
            for (blk, p0, ln, cofs) in segs:
                pt = psTR.tile([P, P], BF16, tag="tr")
                nc.tensor.transpose(pt[p0:p0 + ln, :],
                                    attb[:, cofs:cofs + ln], ident[:],
                                    tile_position=(0, p0))
                st = att_pool.tile([P, P], FP8, tag="attT_sb", bufs=4)
                nc.scalar.copy(st[p0:p0 + ln, :], pt[p0:p0 + ln, :])
                attT.append(st)
            return attT

        def attn_out(x, attT):
            """out_tok = attn^T-weighted sum of U rows; psum=8*out_tok."""
            segs = _segments(x)
            otok = otok_pool.tile([P, C], BF16)
            for ch in range(C // 512):
                cs = slice(ch * 512, (ch + 1) * 512)
                ops = ps512.tile([P, 512], F32, tag="v")
                for k, (blk, p0, ln, cofs) in enumerate(segs):
                    nc.tensor.matmul(
                        ops[:], attT[k][p0:p0 + ln, :],
                        uA[p0:p0 + ln, blk % 4, cs],
                        start=(k == 0), stop=(k == len(segs) - 1),
                        tile_position=(p0, 0))
                nc.scalar.activation(otok[:, cs], ops[:],
                                     mybir.ActivationFunctionType.Copy,
                                     scale=float(1.0 / S_QK))
            return otok

        def phase_b(x, tiles, otok):
            """In-place residual broadcast-add on the resident xb
            chunks of block x, then fp16 store (ACT HWDGE ring so
            stores never head-of-line-block the SP-ring loads).
            t-major layout puts the broadcast on an outer dim, so the
            adds run on the DVE 2x path (~2.2us/chunk)."""
            for j in range(NCH):
                t = tiles[j]
                nc.vector.tensor_tensor(
                    t[:], t[:],
                    otok.unsqueeze(1).broadcast_to((P, TC, C)),
                    op=mybir.AluOpType.add)
                nc.scalar.dma_start(
                    out_d[x * P:(x + 1) * P, j * TC:(j + 1) * TC, :], t[:])

        # Software pipeline, phase B lagged TWO slots behind the loads:
        # at slot i the stores of block i-2 (whose out_tok was computed
        # during slot i-1) begin immediately, so the DMA engines never
        # wait on the current slot's attention chain.
        prev = None        # (x, tiles, qTb): awaiting attention
        pend = None        # (x, tiles, otok): awaiting phase B
        for i in range(NBLK):
            if pend is not None:
                phase_b(*pend)
                pend = None
            tiles_i, tokT_i, posT8_i, posu8_i = load_sum_kv(i)
            if prev is not None:
                x, tiles_x, qTb_x = prev
                attT_x = attn_scores(x, qTb_x)
                proj_u(i, tokT_i, posu8_i)   # PE fills softmax latency
                otok_x = attn_out(x, attT_x)
                pend = (x, tiles_x, otok_x)
            else:
                proj_u(i, tokT_i, posu8_i)
            qTb_i = proj_q(i, tokT_i, posT8_i)
            prev = (i, tiles_i, qTb_i)
        if pend is not None:
            phase_b(*pend)
        x, tiles_x, qTb_x = prev
        attT_x = attn_scores(x, qTb_x)
        otok_x = attn_out(x, attT_x)
        phase_b(x, tiles_x, otok_x)


_NC = None


def _get_nc():
    global _NC
    if _NC is None:
        _NC = build_nc()
    return _NC


def _prep_in_maps(xb, Wq, Wk, Wv, Wo, pos):
    # device works t-major: [NB, T, C]
    xb_h = np.asarray(xb, np.float32).transpose(0, 1, 3, 2).astype(
        np.float16)
    wq8 = (np.asarray(Wq, np.float32) * (S_W / T)).astype(NPF8)
    wk8 = (np.asarray(Wk, np.float32) * (S_W / T)).astype(NPF8)
    wvo = (np.asarray(Wv, np.float32) / T) @ np.asarray(Wo, np.float32)
    wvo8 = (wvo * S_WVO).astype(NPF8)
    posu8 = ((np.asarray(pos, np.float32) @ np.asarray(Wo, np.float32))
             * S_QK).astype(NPF8)
    post8 = np.ascontiguousarray(
        (np.asarray(pos, np.float32) * S_QK).T).astype(NPF8)
    mask_h = _MASKS64.astype(ml_dtypes.bfloat16)
    in_maps = []
    for b in range(B):
        in_maps.append({
            "xbh": np.ascontiguousarray(xb_h[b]),
            "wq": wq8, "wk": wk8, "wvo": wvo8,
            "posu": posu8, "post": post8, "mask": mask_h,
        })
    return in_maps


def _post(res):
    out = np.stack([res.results[b]["out"] for b in range(B)], axis=0)
    # [B, NB, T, C] fp16 -> [B, NB, C, T] fp32
    return np.ascontiguousarray(out.transpose(0, 1, 3, 2)).astype(
        np.float32)


def kernel(xb, Wq, Wk, Wv, Wo, pos):
    nc = _get_nc()
    in_maps = _prep_in_maps(xb, Wq, Wk, Wv, Wo, pos)
    res = run_bass_kernel_spmd(nc, in_maps, core_ids=list(range(N_CORES)))
    return _post(res)


def run_profiled(xb, Wq, Wk, Wv, Wo, pos, **kw):
    """Like kernel(), but NTFF-profiled; returns (out, BassKernelResults)."""
    import sys, types
    if "antenv.axon_hooks" not in sys.modules:
        try:
            from trn_agent_boot.trn_boot import _ntff_profile_via_ctypes
            hook = _ntff_profile_via_ctypes('/opt/axon/libaxon_pjrt.so')
            mod = types.ModuleType("antenv.axon_hooks")
            mod.get_axon_ntff_profile_hook = lambda: hook
            mod.set_axon_ntff_profile_hook = lambda h: None
            sys.modules["antenv.axon_hooks"] = mod
            import concourse.bass_utils as bu
            bu.upload_artifacts = lambda tmpdir: f"local:{tmpdir}"
        except Exception as e:
            print(f"profiling shim unavailable: {e}")
    nc = _get_nc()
    in_maps = _prep_in_maps(xb, Wq, Wk, Wv, Wo, pos)
    res = run_bass_kernel_spmd(nc, in_maps, core_ids=list(range(N_CORES)),
                               trace=True, **kw)
    return _post(res), res
